# revision 2
# baseline (speedup 1.0000x reference)
"""COLoRALinear fused kernel for 8 TRN2 NeuronCores (Bass/Tile).

Computation (per reference):
  base_out   = x @ W^T + b                         [B,S,Do]
  shared_out = (x @ As^T) @ Bs^T * SCALING
  routing    = softmax(mean_s(x) @ task_emb^T)     [B,E]
  task_out   = sum_e routing[b,e] * (x @ Ae^T) @ Be^T * SCALING
  out = base_out + cw*shared_out + (1-cw)*task_out,  cw = sigmoid(collab_w)

Sharding: flatten x to [B*S, Din] = [8192, 2048]; core c owns rows
[c*1024, (c+1)*1024) — all from batch b = c//2.  W and the low-rank
params are replicated.

The routing weights are 8 floats per batch that depend only on
mean_s(x) @ task_emb^T.  The host pass that packs/transposes x already
touches every element, so routing is computed there and folded into the
per-core C2 matrix (expert rows pre-scaled by (1-cw)*SCALING*r_e).
This removes the on-device collective (a ~42us barrier+AllReduce
latency chain), the deferred-chunk staging it forced, and the Emap
scale matmul.

On-core algorithm (all matmuls fp16 with fp32 PSUM accumulation):
  stage1: u[72, m] = Aall @ x_shard^T where Aall stacks
          [shared_A (8); expert_A (64)], interleaved i-plane by i-plane
          with the first six base-GEMM chunks so the PE starts real
          work as soon as plane 0 arrives (group0 uses 6 PSUM banks,
          stage1 the remaining 2).
  u16:    psum -> fp16 SBUF cast; row 72 = ones (bias row via DMA).
  chunks: each output chunk = 16 accumulating base matmuls + a 17th
          accumulating matmul u16^T @ C2 adding shared+task+bias.
"""

import numpy as np

import concourse.bass as bass
import concourse.mybir as mybir
import concourse.tile as tile
from concourse import bacc
from concourse.bass import ts
from concourse.bass_utils import run_bass_kernel_spmd

# Problem shapes (hardcoded per spec)
B, S, DIN, DOUT = 4, 2048, 2048, 2048
E, R = 8, 8
SCALING = 16.0 / 8.0
N_CORES = 8
M_CORE = B * S // N_CORES          # 1024 rows per core
P = 128                            # partitions
KT = DIN // P                      # 16 contraction chunks
NOC = DOUT // 512                  # 4 output chunks of 512
NMT = M_CORE // P                  # 8 m-tiles of 128
AW = 72                            # rows of A-stack: 8 shared + 64 expert
CW = 73                            # rows of C2: 8 shared + 64 expert + 1 bias
WARMUP_MM = 24                     # junk matmuls to flip the PE clock-gate early

# fp16 runs at the same TensorE rate as bf16 with 10 mantissa bits; all
# operand magnitudes here are in [1e-3, 5] so no overflow/subnormal risk
BF16 = np.float16

# set by test.py for profiling
TRACE = False
LAST_RESULT = None

_cached = None

# group0: first six chunks, i-streamed against the DMA (leaves 2 PSUM
# banks for stage1's u accumulation)
GROUP0 = [(mt, oc) for mt in (0, 1) for oc in (0, 1, 2)]


def _build_nc():
    nc = bacc.Bacc(
        "TRN2",
        target_bir_lowering=False,
        debug=False,
        num_devices=N_CORES,
    )
    BF = mybir.dt.float16
    F32 = mybir.dt.float32

    # host-packed layouts: partition-major so every DMA reads large
    # contiguous runs per partition
    xT_d = nc.dram_tensor("xT", [DIN, M_CORE], BF, kind="ExternalInput")
    WT_d = nc.dram_tensor("WT", [P, NOC, KT, 512], BF, kind="ExternalInput")
    AallT_d = nc.dram_tensor("AallT", [P, KT, AW], BF, kind="ExternalInput")
    C2_d = nc.dram_tensor("C2", [CW, DOUT], BF, kind="ExternalInput")
    out_d = nc.dram_tensor("out", [M_CORE, DOUT], F32, kind="ExternalOutput")
    ones_d = nc.dram_tensor("ones", [M_CORE], BF, kind="ExternalInput")

    with tile.TileContext(nc) as tc:
        with (
            tc.tile_pool(name="consts", bufs=1) as consts,
            tc.tile_pool(name="small", bufs=1) as small,
            tc.tile_pool(name="pmm", bufs=6, space="PSUM") as pmm,
            tc.tile_pool(name="psmall", bufs=1, space="PSUM") as psmall,
            tc.tile_pool(name="outp", bufs=3) as outp,
        ):
            # ---- input loads ----
            # One FIFO HW queue services all sync-engine DMAs, so issue
            # order == arrival order.  i-major interleaving: plane i's x
            # chunk plus W slabs for oc 0..2 arrive together so group0
            # can consume plane i right away.  oc3 slabs follow, then C2.
            AallT_sb = consts.tile([P, KT, AW], BF)
            nc.sync.dma_start(AallT_sb[:, :, :], AallT_d[:, :, :])
            xT_sb = consts.tile([P, KT, M_CORE], BF)
            WT_sb = consts.tile([P, NOC, KT, 512], BF)

            for i in range(KT):
                nc.sync.dma_start(xT_sb[:, i, :], xT_d[ts(i, P), :])
                for oc in range(3):
                    nc.sync.dma_start(
                        WT_sb[:, oc, i, :], WT_d[:, oc, i, :]
                    )
            for q in range(4):
                nc.sync.dma_start(
                    WT_sb[:, 3, ts(q, 4), :], WT_d[:, 3, ts(q, 4), :]
                )
            C2_sb = consts.tile([CW, DOUT], BF)
            nc.sync.dma_start(C2_sb[:], C2_d[:, :])

            # ---- PE warmup ----
            # Depends only on the first (small) AallT DMA; ramps the PE
            # p-state while plane 0 is still in flight.  Never read.
            warm_ps = pmm.tile([P, 512], mybir.dt.float32, tag="ps")
            for w in range(WARMUP_MM):
                nc.tensor.matmul(
                    warm_ps[0:AW, 0:AW],
                    AallT_sb[:, w % KT, :],
                    AallT_sb[:, (w * 7 + 3) % KT, :],
                    start=True,
                    stop=True,
                )

            # ---- group0 + stage1, i-streamed ----
            ps6 = {}
            for mt, oc in GROUP0:
                g0ps = pmm.tile([P, 512], mybir.dt.float32, tag="ps",
                                name=f"g0ps_{mt}_{oc}")
                ps6[(mt, oc)] = g0ps
            u_ps_a = psmall.tile([AW, 512], mybir.dt.float32, tag="u_ps")
            u_ps_b = psmall.tile([AW, 512], mybir.dt.float32, tag="u_ps2")
            u_ps = {0: u_ps_a, 1: u_ps_b}
            for i in range(KT):
                for mt, oc in GROUP0:
                    nc.tensor.matmul(
                        ps6[(mt, oc)][:, :],
                        xT_sb[:, i, ts(mt, P)],
                        WT_sb[:, oc, i, :],
                        start=(i == 0),
                        stop=False,
                    )
                for h in range(2):
                    nc.tensor.matmul(
                        u_ps[h][:, :],
                        AallT_sb[:, i, :],
                        xT_sb[:, i, ts(h, 512)],
                        start=(i == 0),
                        stop=(i == KT - 1),
                    )

            # ---- u -> fp16, bias ones row ----
            u16 = small.tile([CW, M_CORE], BF)
            nc.gpsimd.dma_start(u16[AW : AW + 1, :], ones_d[:])
            for h in range(2):
                nc.vector.tensor_copy(u16[0:AW, ts(h, 512)], u_ps[h][:, :])

            def finish_chunk(mt, oc, ps):
                # 17th accumulating matmul: shared+task low-rank + bias
                nc.tensor.matmul(
                    ps[:],
                    u16[0:CW, ts(mt, P)],
                    C2_sb[0:CW, ts(oc, 512)],
                    start=False,
                    stop=True,
                )
                ob = outp.tile([P, 512], F32, tag="ob")
                nc.vector.tensor_copy(ob[:], ps[:])
                nc.sync.dma_start(out_d[ts(mt, P), ts(oc, 512)], ob[:])

            for mt, oc in GROUP0:
                finish_chunk(mt, oc, ps6[(mt, oc)])

            # ---- remaining chunks ----
            # (mt0-1, oc3) last: their W slabs are the final input loads
            rest = [(mt, oc) for mt in range(2, NMT) for oc in range(NOC)]
            rest += [(0, 3), (1, 3)]
            for mt, oc in rest:
                ps = pmm.tile([P, 512], mybir.dt.float32, tag="ps")
                for i in range(KT):
                    nc.tensor.matmul(
                        ps[:],
                        xT_sb[:, i, ts(mt, P)],
                        WT_sb[:, oc, i, :],
                        start=(i == 0),
                        stop=False,
                    )
                finish_chunk(mt, oc, ps)

    nc.compile()
    return nc


def _prep_inputs(x, base_W, base_b, shared_A, shared_B, expert_A, expert_B,
                 task_emb, collab_w):
    f = np.float32
    x = np.asarray(x, dtype=f).reshape(B * S, DIN)
    base_W = np.asarray(base_W, dtype=f)
    base_b = np.asarray(base_b, dtype=f)
    shared_A = np.asarray(shared_A, dtype=f)
    shared_B = np.asarray(shared_B, dtype=f)
    expert_A = np.asarray(expert_A, dtype=f)
    expert_B = np.asarray(expert_B, dtype=f)
    task_emb = np.asarray(task_emb, dtype=f)
    cw = float(1.0 / (1.0 + np.exp(-np.asarray(collab_w, dtype=np.float64))))

    # routing on host: 8 floats per batch, folded into C2 expert rows
    x_mean = x.reshape(B, S, DIN).mean(axis=1)               # [B, Din]
    logits = x_mean @ task_emb.T                             # [B, E]
    m = logits.max(axis=1, keepdims=True)
    ex = np.exp(logits - m)
    routing = ex / ex.sum(axis=1, keepdims=True)             # [B, E]

    # partition-major packed layouts (large contiguous DMA bursts);
    # cast to fp16 BEFORE the transposed copies to halve host memcpy bytes
    # WT[p, oc, i, j] = base_W.T[i*128+p, oc*512+j]
    WT = np.ascontiguousarray(
        base_W.astype(BF16).T.reshape(KT, P, NOC, 512).transpose(1, 2, 0, 3)
    )                                                                # [P,NOC,KT,512]
    # A-stack rows: 0..7 shared, 8..71 expert
    A_all = np.concatenate([shared_A, expert_A.reshape(E * R, DIN)], axis=0)
    # AallT[p, i, a] = A_all[a, i*128+p]
    AallT = np.ascontiguousarray(
        A_all.T.reshape(KT, P, AW).transpose(1, 0, 2)
    ).astype(BF16)                                                   # [P,KT,AW]

    # C2 rows align with u16 rows; row 72 = bias via ones-row.
    # Expert rows carry the per-batch routing weight.
    eB = expert_B.transpose(0, 2, 1).reshape(E * R, DOUT)            # [(e,r),Do]
    C2s = []
    for b in range(B):
        C2 = np.empty((CW, DOUT), dtype=f)
        C2[0:8] = shared_B.T * (cw * SCALING)
        scale_e = ((1.0 - cw) * SCALING) * routing[b]                # [E]
        C2[8:72] = eB * np.repeat(scale_e, R)[:, None]
        C2[72] = base_b
        C2s.append(C2.astype(BF16))

    ones = np.ones((M_CORE,), dtype=BF16)

    x16 = x.astype(BF16)
    in_maps = []
    for c in range(N_CORES):
        xT = np.ascontiguousarray(x16[c * M_CORE : (c + 1) * M_CORE].T)
        in_maps.append(
            {"xT": xT, "WT": WT, "AallT": AallT, "C2": C2s[c // 2],
             "ones": ones}
        )
    return in_maps


def kernel(**inputs):
    global _cached, LAST_RESULT
    if _cached is None:
        _cached = _build_nc()
    nc = _cached
    in_maps = _prep_inputs(**inputs)
    res = run_bass_kernel_spmd(
        nc, in_maps, core_ids=list(range(N_CORES)), trace=TRACE
    )
    LAST_RESULT = res
    out = np.concatenate(
        [res.results[c]["out"] for c in range(N_CORES)], axis=0
    ).reshape(B, S, DOUT)
    return np.ascontiguousarray(out.astype(np.float32))


# revision 3
# speedup vs baseline: 1.1135x; 1.1135x over previous
"""COLoRALinear fused kernel for 8 TRN2 NeuronCores (Bass/Tile).

Computation (per reference):
  base_out   = x @ W^T + b                         [B,S,Do]
  shared_out = (x @ As^T) @ Bs^T * SCALING
  routing    = softmax(mean_s(x) @ task_emb^T)     [B,E]
  task_out   = sum_e routing[b,e] * (x @ Ae^T) @ Be^T * SCALING
  out = base_out + cw*shared_out + (1-cw)*task_out,  cw = sigmoid(collab_w)

Sharding: flatten x to [B*S, Din] = [8192, 2048]; core c owns rows
[c*1024, (c+1)*1024) — all from batch b = c//2.  W and the low-rank
params are replicated.

The routing weights are 8 floats per batch depending only on
mean_s(x) @ task_emb^T.  The host pass that packs/transposes x already
touches every element, so routing is computed there and folded into the
per-core C2 matrix (expert rows pre-scaled by (1-cw)*SCALING*r_e).
This removes the on-device collective (a ~42us barrier+AllReduce
latency chain) and the deferred-chunk staging it forced; every chunk
fuses its low-rank epilogue immediately.

On-core algorithm (all matmuls fp16 with fp32 PSUM accumulation):
  stage1: u[72, m] = Aall @ x_shard^T, x-DMA-paced, junk matmuls
          filling the pacing gaps to hold the PE clock at full speed
          (a multi-us PE idle triggers a ~20us half-clock HAM window).
  u16:    psum -> fp16 SBUF cast; row 72 = ones (bias row via DMA).
  chunks: 16 accumulating base matmuls + a 17th accumulating matmul
          u16^T @ C2 adding shared+task+bias, then DVE evac + DMA out.
"""

import numpy as np

import concourse.bass as bass
import concourse.mybir as mybir
import concourse.tile as tile
from concourse import bacc
from concourse.bass import ts
from concourse.bass_utils import run_bass_kernel_spmd

# Problem shapes (hardcoded per spec)
B, S, DIN, DOUT = 4, 2048, 2048, 2048
E, R = 8, 8
SCALING = 16.0 / 8.0
N_CORES = 8
M_CORE = B * S // N_CORES          # 1024 rows per core
P = 128                            # partitions
KT = DIN // P                      # 16 contraction chunks
NOC = DOUT // 512                  # 4 output chunks of 512
NMT = M_CORE // P                  # 8 m-tiles of 128
AW = 72                            # rows of A-stack: 8 shared + 64 expert
CW = 73                            # rows of C2: 8 shared + 64 expert + 1 bias
WQ = 4                             # WT slab split: KT/WQ i-chunks per DMA
WARMUP_MM = 40                     # junk matmuls to flip the PE clock-gate early

BF16 = np.float16

# set by test.py for profiling
TRACE = False
LAST_RESULT = None

_cached = None


def _build_nc():
    nc = bacc.Bacc(
        "TRN2",
        target_bir_lowering=False,
        debug=False,
        num_devices=N_CORES,
    )
    BF = mybir.dt.float16
    F32 = mybir.dt.float32

    # host-packed layouts: partition-major so every DMA reads large
    # contiguous runs per partition
    xT_d = nc.dram_tensor("xT", [DIN, M_CORE], BF, kind="ExternalInput")
    WT_d = nc.dram_tensor("WT", [P, NOC, KT, 512], BF, kind="ExternalInput")
    AallT_d = nc.dram_tensor("AallT", [P, KT, AW], BF, kind="ExternalInput")
    C2_d = nc.dram_tensor("C2", [CW, DOUT], BF, kind="ExternalInput")
    out_d = nc.dram_tensor("out", [M_CORE, DOUT], F32, kind="ExternalOutput")
    ones_d = nc.dram_tensor("ones", [M_CORE], BF, kind="ExternalInput")

    with tile.TileContext(nc) as tc:
        with (
            tc.tile_pool(name="consts", bufs=1) as consts,
            tc.tile_pool(name="small", bufs=1) as small,
            tc.tile_pool(name="pmm", bufs=6, space="PSUM") as pmm,
            tc.tile_pool(name="psmall", bufs=1, space="PSUM") as psmall,
            tc.tile_pool(name="outp", bufs=3) as outp,
        ):
            # ---- input loads ----
            # One FIFO HW queue services all sync-engine DMAs, so issue
            # order == arrival order.  Interleave xT with WT's first slabs
            # so the base loop can start right after stage-1 drains; C2
            # lands just before the first chunk's epilogue needs it.
            AallT_sb = consts.tile([P, KT, AW], BF)
            nc.sync.dma_start(AallT_sb[:, :, :], AallT_d[:, :, :])
            xT_sb = consts.tile([P, KT, M_CORE], BF)
            WT_sb = consts.tile([P, NOC, KT, 512], BF)

            def wt_load(oc, iq):
                nc.sync.dma_start(
                    WT_sb[:, oc, iq * WQ : (iq + 1) * WQ, :],
                    WT_d[:, oc, iq * WQ : (iq + 1) * WQ, :],
                )

            for i in range(0, 8):
                nc.sync.dma_start(xT_sb[:, i, :], xT_d[ts(i, P), :])
            wt_load(0, 0)
            wt_load(0, 1)
            for i in range(8, KT):
                nc.sync.dma_start(xT_sb[:, i, :], xT_d[ts(i, P), :])
            wt_load(0, 2)
            wt_load(0, 3)
            C2_sb = consts.tile([CW, DOUT], BF)
            nc.sync.dma_start(C2_sb[:], C2_d[:, :])
            for oc in range(1, NOC):
                for iq in range(KT // WQ):
                    wt_load(oc, iq)

            # bias ones row via gpsimd SWDGE (off the bulk HW queue);
            # engine ops need 32-aligned partition bases, DMA does not
            u16 = small.tile([CW, M_CORE], BF)
            nc.gpsimd.dma_start(u16[AW : AW + 1, :], ones_d[:])

            # ---- PE warmup ----
            # Depends only on the (small, first) AallT DMA; keeps the PE
            # busy before stage-1 so the HAM clock-gate reaches 2.4GHz
            # early.  Results are never read.
            warm_ps = pmm.tile([P, 512], mybir.dt.float32, tag="ps")

            def junk_mm(w):
                nc.tensor.matmul(
                    warm_ps[0:AW, 0:AW],
                    AallT_sb[:, w % KT, :],
                    AallT_sb[:, (w * 7 + 3) % KT, :],
                    start=True,
                    stop=True,
                )

            for w in range(WARMUP_MM):
                junk_mm(w)

            # ---- stage 1: u[72, m], both m-halves interleaved per i so
            # the PE duty cycle stays high while xT tiles stream in;
            # junk fillers plug the remaining DMA-pacing gaps ----
            u_ps_a = psmall.tile([AW, 512], mybir.dt.float32, tag="u_ps")
            u_ps_b = psmall.tile([AW, 512], mybir.dt.float32, tag="u_ps2")
            u_ps = {0: u_ps_a, 1: u_ps_b}
            for i in range(KT):
                for h in range(2):
                    nc.tensor.matmul(
                        u_ps[h][:, :],
                        AallT_sb[:, i, :],
                        xT_sb[:, i, ts(h, 512)],
                        start=(i == 0),
                        stop=(i == KT - 1),
                    )
                junk_mm(2 * i)
                junk_mm(2 * i + 1)
            for h in range(2):
                nc.vector.tensor_copy(u16[0:AW, ts(h, 512)], u_ps[h][:, :])

            # ---- main loop: base matmul + fused epilogue ----
            for oc in range(NOC):
                for mt in range(NMT):
                    ps = pmm.tile([P, 512], mybir.dt.float32, tag="ps")
                    for i in range(KT):
                        nc.tensor.matmul(
                            ps[:],
                            xT_sb[:, i, ts(mt, P)],
                            WT_sb[:, oc, i, :],
                            start=(i == 0),
                            stop=False,
                        )
                    # 17th accumulating matmul: shared+task low-rank + bias
                    nc.tensor.matmul(
                        ps[:],
                        u16[0:CW, ts(mt, P)],
                        C2_sb[0:CW, ts(oc, 512)],
                        start=False,
                        stop=True,
                    )
                    ob = outp.tile([P, 512], F32, tag="ob")
                    nc.vector.tensor_copy(ob[:], ps[:])
                    nc.sync.dma_start(out_d[ts(mt, P), ts(oc, 512)], ob[:])

    nc.compile()
    return nc


def _prep_inputs(x, base_W, base_b, shared_A, shared_B, expert_A, expert_B,
                 task_emb, collab_w):
    f = np.float32
    x = np.asarray(x, dtype=f).reshape(B * S, DIN)
    base_W = np.asarray(base_W, dtype=f)
    base_b = np.asarray(base_b, dtype=f)
    shared_A = np.asarray(shared_A, dtype=f)
    shared_B = np.asarray(shared_B, dtype=f)
    expert_A = np.asarray(expert_A, dtype=f)
    expert_B = np.asarray(expert_B, dtype=f)
    task_emb = np.asarray(task_emb, dtype=f)
    cw = float(1.0 / (1.0 + np.exp(-np.asarray(collab_w, dtype=np.float64))))

    # routing on host: 8 floats per batch, folded into C2 expert rows
    x_mean = x.reshape(B, S, DIN).mean(axis=1)               # [B, Din]
    logits = x_mean @ task_emb.T                             # [B, E]
    m = logits.max(axis=1, keepdims=True)
    ex = np.exp(logits - m)
    routing = ex / ex.sum(axis=1, keepdims=True)             # [B, E]

    # partition-major packed layouts (large contiguous DMA bursts);
    # cast to fp16 BEFORE the transposed copies to halve host memcpy bytes
    # WT[p, oc, i, j] = base_W.T[i*128+p, oc*512+j]
    WT = np.ascontiguousarray(
        base_W.astype(BF16).T.reshape(KT, P, NOC, 512).transpose(1, 2, 0, 3)
    )                                                                # [P,NOC,KT,512]
    # A-stack rows: 0..7 shared, 8..71 expert
    A_all = np.concatenate([shared_A, expert_A.reshape(E * R, DIN)], axis=0)
    # AallT[p, i, a] = A_all[a, i*128+p]
    AallT = np.ascontiguousarray(
        A_all.T.reshape(KT, P, AW).transpose(1, 0, 2)
    ).astype(BF16)                                                   # [P,KT,AW]

    # C2 rows align with u16 rows; row 72 = bias via ones-row.
    # Expert rows carry the per-batch routing weight.
    eB = expert_B.transpose(0, 2, 1).reshape(E * R, DOUT)            # [(e,r),Do]
    C2s = []
    for b in range(B):
        C2 = np.empty((CW, DOUT), dtype=f)
        C2[0:8] = shared_B.T * (cw * SCALING)
        scale_e = ((1.0 - cw) * SCALING) * routing[b]                # [E]
        C2[8:72] = eB * np.repeat(scale_e, R)[:, None]
        C2[72] = base_b
        C2s.append(C2.astype(BF16))

    ones = np.ones((M_CORE,), dtype=BF16)

    x16 = x.astype(BF16)
    in_maps = []
    for c in range(N_CORES):
        xT = np.ascontiguousarray(x16[c * M_CORE : (c + 1) * M_CORE].T)
        in_maps.append(
            {"xT": xT, "WT": WT, "AallT": AallT, "C2": C2s[c // 2],
             "ones": ones}
        )
    return in_maps


def kernel(**inputs):
    global _cached, LAST_RESULT
    if _cached is None:
        _cached = _build_nc()
    nc = _cached
    in_maps = _prep_inputs(**inputs)
    res = run_bass_kernel_spmd(
        nc, in_maps, core_ids=list(range(N_CORES)), trace=TRACE
    )
    LAST_RESULT = res
    out = np.concatenate(
        [res.results[c]["out"] for c in range(N_CORES)], axis=0
    ).reshape(B, S, DOUT)
    return np.ascontiguousarray(out.astype(np.float32))
